# revision 43
# baseline (speedup 1.0000x reference)
"""NRI edge layer (gnn message passing) on 8 TRN2 NeuronCores.

Strategy: shard the destination-node axis d (256 -> 32 per core). Each core
computes out[d_loc, :] completely (needs all source nodes s); the unshard is a
pure concatenation -- no collective.

Per-core pipeline (all loops fully unrolled, Tile framework):
  fsT[x, s] = node @ W0s + b0  (PE fp32, stored bf16)   fdT[x, d_loc] likewise
  per d-group of 4, per k-pair:
      z[x2, s]    = relu(fsT2 + fdT2[:, d])   (DVE/GPSIMD tensor_scalar, bf16)
      psum1[y2,s] = W1pair.T @ z              (PE, bf16, 2 d per matmul)
      acts[y2, s] = relu(psum1 + b1)          (ACT/DVE, bf16)
  actsT = DMA-transpose(acts, whole d-group)  (DMA xbar, 2 queues round-robin)
  out[d, y] += w[s,d,k] . actsT[s, y]         (PE col-matvec, PSUM accumulate,
                                               pipelined one d-group behind)
"""

import numpy as np

import concourse.bacc as bacc
import concourse.tile as tile
import concourse.mybir as mybir
from concourse import bass_utils

dt = mybir.dt
AF = mybir.ActivationFunctionType
ALU = mybir.AluOpType

N = 256          # nodes
D = 128          # node embedding dim
H = 64           # hidden/out dim
KM = 9           # number of MLPs (8 edge + 1 mask)
NCORES = 8
DLOC = N // NCORES   # dest nodes per core = 32
KP = 5               # k-pairs: (0,1)(2,3)(4,5)(6,7)(8,-)
DG = 8               # d-groups per core
GS = DLOC // DG      # 4 dest nodes per group

_CACHE = {}


def _build():
    nc = bacc.Bacc("TRN2", target_bir_lowering=False, debug=False,
                   num_devices=NCORES)

    node = nc.dram_tensor("node", [N, D], dt.float32, kind="ExternalInput")
    node_d = nc.dram_tensor("node_d", [DLOC, D], dt.float32, kind="ExternalInput")
    ident = nc.dram_tensor("ident", [128, 128], dt.float32, kind="ExternalInput")
    w0s = nc.dram_tensor("w0s", [KM, D, H], dt.float32, kind="ExternalInput")
    w0d = nc.dram_tensor("w0d", [KM, D, H], dt.float32, kind="ExternalInput")
    b0 = nc.dram_tensor("b0", [KM, H], dt.float32, kind="ExternalInput")
    w1 = nc.dram_tensor("w1", [KM, H, H], dt.float32, kind="ExternalInput")
    b1 = nc.dram_tensor("b1", [KM, H], dt.float32, kind="ExternalInput")
    edge_s = nc.dram_tensor("edge_s", [N, DLOC, KM - 1], dt.float32,
                            kind="ExternalInput")
    mask_s = nc.dram_tensor("mask_s", [N, DLOC], dt.float32, kind="ExternalInput")
    out = nc.dram_tensor("out", [DLOC, H], dt.float32, kind="ExternalOutput")

    with tile.TileContext(nc) as tc:
        with tc.tile_pool(name="const", bufs=1) as cpool:
            # ---------------- constants / precompute ----------------
            id_t = cpool.tile([128, 128], dt.float32)
            nc.sync.dma_start(id_t[:], ident[:])

            node_sb = cpool.tile([128, 2, 128], dt.float32)
            nc.sync.dma_start(node_sb[:], node[:].rearrange("(t p) x -> p t x", p=128))
            noded_sb = cpool.tile([DLOC, 128], dt.float32)
            nc.sync.dma_start(noded_sb[:], node_d[:])

            w0s_sb = cpool.tile([128, KM, H], dt.float32)
            nc.sync.dma_start(w0s_sb[:], w0s[:].rearrange("k x y -> x k y"))
            w0d_sb = cpool.tile([128, KM, H], dt.float32)
            nc.sync.dma_start(w0d_sb[:], w0d[:].rearrange("k x y -> x k y"))

            # b0/b1 pair-rows [5, 128] then PE-transpose to [128, 5]
            b0p_st = cpool.tile([KP, 128], dt.float32)
            b1p_st = cpool.tile([KP, 128], dt.float32)
            for st_t, src in ((b0p_st, b0), (b1p_st, b1)):
                nc.gpsimd.memset(st_t[:], 0.0)
                nc.sync.dma_start(
                    st_t[0:4, :], src[0:8, :].rearrange("(kp i) y -> kp (i y)", i=2))
                nc.sync.dma_start(st_t[4:5, 0:H], src[8:9, :])
            b0_sb = cpool.tile([128, KP], dt.float32)
            b1_sb = cpool.tile([128, KP], dt.float32)

            fsT2 = cpool.tile([128, KP, N], dt.bfloat16)
            fdT2 = cpool.tile([128, KP, DLOC], dt.float32)

            with tc.tile_pool(name="pspre", bufs=2, space="PSUM") as pspre:
                for st_t, dst in ((b0p_st, b0_sb), (b1p_st, b1_sb)):
                    pb = pspre.tile([128, KP], dt.float32, tag="pre")
                    nc.tensor.transpose(pb[:], st_t[:], id_t[0:KP, 0:KP])
                    nc.vector.tensor_copy(dst[:], pb[:])

                # nodeT [x=128, n=256] via PE transpose
                nodeT = cpool.tile([128, N], dt.bfloat16)
                for t in range(2):
                    pt = pspre.tile([128, 128], dt.float32, tag="pre")
                    nc.tensor.transpose(pt[:], node_sb[:, t, :], id_t[:])
                    nc.scalar.copy(nodeT[:, t * 128:(t + 1) * 128], pt[:])
                nodeTd = cpool.tile([128, DLOC], dt.bfloat16)
                ptd = pspre.tile([128, DLOC], dt.float32, tag="pre")
                nc.tensor.transpose(ptd[:], noded_sb[:], id_t[0:DLOC, 0:DLOC])
                nc.scalar.copy(nodeTd[:], ptd[:])

                w0s_bb = cpool.tile([128, KM, H], dt.bfloat16)
                nc.vector.tensor_copy(w0s_bb[:], w0s_sb[:])
                w0d_bb = cpool.tile([128, KM, H], dt.bfloat16)
                nc.vector.tensor_copy(w0d_bb[:], w0d_sb[:])
                # fsT2 = node@W0s + b0 (pair-stacked, bf16); fdT2 = node_d@W0d
                for kp in range(KP):
                    pfs = pspre.tile([128, N], dt.float32, tag="pre")
                    pfd = pspre.tile([128, DLOC], dt.float32, tag="pre")
                    for i in (0, 1):
                        k = kp * 2 + i
                        if k >= KM:
                            continue
                        nc.tensor.matmul(pfs[i * H:(i + 1) * H, :],
                                         w0s_bb[:, k, :], nodeT[:])
                        nc.tensor.matmul(pfd[i * H:(i + 1) * H, :],
                                         w0d_bb[:, k, :], nodeTd[:])
                    if kp == KP - 1:  # zero the unused halves
                        nc.gpsimd.memset(fsT2[H:128, kp, :], 0.0)
                        nc.gpsimd.memset(fdT2[H:128, kp, :], 0.0)
                    lim = H if kp == KP - 1 else 128
                    nc.vector.tensor_scalar(fsT2[0:lim, kp, :], pfs[0:lim, :],
                                            b0_sb[0:lim, kp:kp + 1], None,
                                            ALU.add)
                    nc.vector.tensor_copy(fdT2[0:lim, kp, :], pfd[0:lim, :])

            # W1 block-diag pairs, bf16: [128, 5, 128]
            w1_st = cpool.tile([H, KM, H], dt.float32)
            nc.sync.dma_start(w1_st[:], w1[:].rearrange("k x y -> x k y"))
            w1p_sb = cpool.tile([128, KP, 128], dt.bfloat16)
            nc.gpsimd.memset(w1p_sb[:], 0.0)
            for k in range(KM):
                i, kp = k % 2, k // 2
                nc.vector.tensor_copy(
                    w1p_sb[i * H:(i + 1) * H, kp, i * H:(i + 1) * H],
                    w1_st[:, k, :])

            # edge weights + mask, bf16 [128, 2, .]
            w8_f = cpool.tile([128, 2, DLOC * (KM - 1)], dt.float32)
            nc.sync.dma_start(
                w8_f[:], edge_s[:].rearrange("(t p) d k -> p t (d k)", p=128))
            w8_sb = cpool.tile([128, 2, DLOC * (KM - 1)], dt.bfloat16)
            nc.vector.tensor_copy(w8_sb[:], w8_f[:])
            mask_f = cpool.tile([128, 2, DLOC], dt.float32)
            nc.sync.dma_start(mask_f[:], mask_s[:].rearrange("(t p) d -> p t d", p=128))
            mask_sb = cpool.tile([128, 2, DLOC], dt.bfloat16)
            nc.vector.tensor_copy(mask_sb[:], mask_f[:])

            # ---------------- main loop ----------------
            with (
                tc.tile_pool(name="zp", bufs=10) as zpool,
                tc.tile_pool(name="ap", bufs=4) as apool,
                tc.tile_pool(name="tp", bufs=4) as tpool,
                tc.tile_pool(name="rp", bufs=2) as rpool,
                tc.tile_pool(name="ps1", bufs=2, space="PSUM") as ps1,
                tc.tile_pool(name="psout", bufs=1, space="PSUM") as psout,
            ):
                out_sb = cpool.tile([128, DG, 128], dt.float32)

                def finals(kq, pq, dg, rhs_of):
                    col0 = (kq % 4) * 512 + (kq // 4) * 128
                    for j in range(GS):
                        d = dg * GS + j
                        n_mm = 2 * len(pq)
                        seq = 0
                        for st in range(2):
                            rhs = rhs_of(j, st)
                            for i, k in enumerate(pq):
                                if k == KM - 1:
                                    wcol = mask_sb[:, st, d:d + 1]
                                else:
                                    wcol = w8_sb[:, st,
                                                 d * (KM - 1) + k:
                                                 d * (KM - 1) + k + 1]
                                nc.tensor.matmul(
                                    p_out[(j % 2) * 64:(j % 2) * 64 + 1,
                                          col0 + (j // 2) * 64:
                                          col0 + (j // 2) * 64 + 64],
                                    wcol, rhs[:, i * H:(i + 1) * H],
                                    start=(seq == 0), stop=(seq == n_mm - 1))
                                seq += 1

                for dg in range(DG):
                    p_out = psout.tile([128, 2048], dt.float32)
                    acts = None
                    for kp in range(KP):
                        pair = [k for k in (kp * 2, kp * 2 + 1) if k < KM]
                        if kp == KP - 1:
                            acts = apool.tile([128, 1, GS, N], dt.bfloat16,
                                              tag="acts4")
                            ai = 0
                        elif kp == 0:
                            acts = apool.tile([128, 4, GS, N], dt.bfloat16,
                                              tag="acts03")
                            ai = 0
                        else:
                            ai = kp
                        ps = ps1.tile([128, GS, N], dt.float32)
                        for jj in range(GS // 2):
                            z2 = zpool.tile([128, 2, N], dt.bfloat16)
                            for u in (0, 1):
                                j = jj * 2 + u
                                d = dg * GS + j
                                if (j + kp) % 4 == 3:
                                    nc.scalar.activation(
                                        z2[:, u, :], fsT2[:, kp, :], AF.Relu,
                                        bias=fdT2[:, kp, d:d + 1])
                                else:
                                    nc.vector.tensor_scalar(
                                        z2[:, u, :], fsT2[:, kp, :],
                                        fdT2[:, kp, d:d + 1],
                                        0.0, ALU.add, ALU.max)
                            nc.tensor.matmul(ps[:, jj * 2:jj * 2 + 2, :],
                                             w1p_sb[:, kp, :], z2[:])
                        if kp >= 4:
                            nc.vector.tensor_scalar(
                                acts[:, ai, :, :], ps[:], b1_sb[:, kp:kp + 1],
                                0.0, ALU.add, ALU.max)
                        else:
                            nc.scalar.activation(acts[:, ai, :, :], ps[:],
                                                 AF.Relu,
                                                 bias=b1_sb[:, kp:kp + 1])
                        if kp == KP - 1:
                            actsT4 = tpool.tile([128, 2 * GS, H],
                                                dt.bfloat16, tag="t4")
                            nc.sync.dma_start_transpose(
                                actsT4[:], acts[0:H, 0, :, :])
                            finals(kp, pair, dg,
                                   lambda j, st: actsT4[:, 2 * j + st, :])
                        elif kp == 3:
                            actsT = tpool.tile([128, 4, 2 * GS, 128],
                                               dt.bfloat16, tag="t03")
                            nc.sync.dma_start_transpose(actsT[:], acts[:])
                            for kq in (0, 1, 2, 3):
                                pq = [kq * 2, kq * 2 + 1]
                                finals(kq, pq, dg,
                                       lambda j, st, _q=kq, _t=actsT:
                                       _t[:, _q, 2 * j + st, :])
                    # drain: out_sb[:, dg, :] = sum over the 5 kp partials
                    red = rpool.tile([128, 128], dt.float32)
                    pv = p_out[:].rearrange("p (kp b) -> p kp b", kp=4)
                    pv = pv[:, :, 0:128].rearrange("p kp y -> p y kp")
                    nc.vector.reduce_sum(red[:], pv, axis=mybir.AxisListType.X)
                    nc.vector.tensor_add(out_sb[:, dg, :], red[:],
                                         p_out[:, 128:256])

                out_r = out[:].rearrange("(dg jf jp) y -> jp dg jf y",
                                         jp=2, jf=2)
                for q in range(2):
                    nc.scalar.dma_start(
                        out_r[q:q + 1, :, :, :],
                        out_sb[q * 64:q * 64 + 1, :, :].rearrange(
                            "p dg (jf y) -> p dg jf y", jf=2))

    nc.compile()
    return nc


def kernel(edge_embeddings, node_embeddings, mask,
           l0_weight_from_source, l0_weight_from_dest, l0_bias,
           l1_weight, l1_bias):
    if "nc" not in _CACHE:
        _CACHE["nc"] = _build()
    nc = _CACHE["nc"]

    edge = np.asarray(edge_embeddings, dtype=np.float32)
    node = np.asarray(node_embeddings, dtype=np.float32)
    maskv = np.asarray(mask, dtype=np.float32)
    common = {
        "node": node,
        "ident": np.eye(128, dtype=np.float32),
        "w0s": np.asarray(l0_weight_from_source, dtype=np.float32),
        "w0d": np.asarray(l0_weight_from_dest, dtype=np.float32),
        "b0": np.asarray(l0_bias, dtype=np.float32),
        "w1": np.asarray(l1_weight, dtype=np.float32),
        "b1": np.asarray(l1_bias, dtype=np.float32),
    }
    in_maps = []
    for c in range(NCORES):
        d0 = c * DLOC
        in_maps.append({
            **common,
            "node_d": np.ascontiguousarray(node[d0:d0 + DLOC]),
            "edge_s": np.ascontiguousarray(edge[:, d0:d0 + DLOC, :]),
            "mask_s": np.ascontiguousarray(maskv[:, d0:d0 + DLOC]),
        })
    global last_in_maps
    last_in_maps = in_maps
    res = bass_utils.run_bass_kernel_spmd(nc, in_maps,
                                          core_ids=list(range(NCORES)))
    return np.concatenate([res.results[c]["out"] for c in range(NCORES)],
                          axis=0)
